# revision 56
# baseline (speedup 1.0000x reference)
"""Causal multi-head attention (B=2, T=2048, D=1024, H=16) on 8 trn2 cores.

Sharding: data-parallel over batch (2) x tensor-parallel over heads (4 groups
of 4 heads): core c handles batch c//4, head group c%4. Each core computes
q/k/v projections for its 256 feature columns, causal attention for its 4
heads, and a partial row-parallel output projection. The host sums the 4
partials per batch and adds bo (plus bv@wo, see below).

Numerics/layout strategy (measured rel err 1.603e-2 vs the f32 reference,
HW exec ~97us/rep by repeat-slope, down from the 110.7us baseline):
- Host pre-transposes x to d-major and pre-casts fp8-e4m3: xT8 plus the
  residual xT8lo, so the device does zero transposes.
- Q/K projections run as fp8 DoubleRow matmuls (256-deep contraction).
  Weights are pre-scaled by 8 on the host (folded back out of the softmax
  exp scale) to keep fp8 away from the subnormal range. The V projection
  runs fp8 DoubleRow too, error-compensated with three terms
  (x8@(wv8+wv8lo) + x8lo@wv8, ~0.1% error); the 8x weight prescale cancels
  in the softmax normalize because the denominator ones-column is set to 8.
  q/k/v are stored bf16; scores and p@v run bf16 -> fp32 psum. (fp8 for the
  exp'd scores themselves was measured at +2.3% output error - over the
  gate - so p@v stays bf16.)
- bk is dropped entirely: (q+bq)@(k+bk) differs from (q+bq)@k by a
  per-query constant, which softmax is invariant to.
- bv is dropped on-device: normalized attention rows sum to 1, so
  attn@(1 x bv)@wo == bv@wo, a constant row the host folds into bo.
- The causal mask costs nothing downstream of exp: a single 128-col PE
  matmul (identb @ mb16) preloads -1e30 into the diag psum triangle before
  the scores accumulate, so exp writes exact zeros and p@v depends ONLY on
  the exp. Diagonal matmuls skip the non-causal junk columns.
- The exp stream on the scalar (Act) engine is a ~76us/rep floor; the PE is
  the binding engine at ~93us/rep. All act-independent PE work (q/k/v
  projections, output projection) is cut into ~0.5-0.9us closures and
  pumped between attention chunks so neither engine starves: tg2/3
  projections fill the (0,1)-group heads; the previous rep's tail
  out-projection, this rep's rows-0:1024 out-projection and the NEXT rep's
  tg0/1 projections fill the (2,3)-group heads. On the last rep the tail
  out-projection fires via post_norm hooks as each group's final norm
  lands, shortening the drain.
- DMA loads are SP-triggered (a dma_start's sem waits hold the issuing
  sequencer; on Act that would head-of-line block the exp stream). Output
  stores stage through SBUF in f32 on the DVE (GPSIMD cannot touch PSUM and
  DMA cannot read it) - f32 stores also drop the bf16 store rounding.
- Startup is ordered x8(tg0/1) + q/k weights first, bulky wv8/wo consts
  after, so the first projection matmuls start ~3us in. Persistent state is
  parity-double-buffered so consecutive reps pipeline.
"""

import sys

if "/opt/trn_rl_repo" not in sys.path:
    sys.path.insert(0, "/opt/trn_rl_repo")

import numpy as np
import ml_dtypes

import concourse.bass as bass
import concourse.mybir as mybir
import concourse.tile as tile
from concourse import bacc

F32 = mybir.dt.float32
BF16 = mybir.dt.bfloat16
F8 = mybir.dt.float8e4
EXP = mybir.ActivationFunctionType.Exp
DR = mybir.MatmulPerfMode.DoubleRow

B, T, D, H, HD = 2, 2048, 1024, 16, 64
SCALE = float(D) ** -0.5  # module scales by d_model^-0.5
NCORES = 8
HPC = 4  # heads per core
JS = HPC * HD  # 256 feature columns per core
NT = T // 128  # 16 t-chunks
ND = D // 128  # 8 d-chunks
WS = 8.0  # fp8 weight prescale, folded out of the exp scale
SCALE_EXP = SCALE / (WS * WS)

NP_BF16 = ml_dtypes.bfloat16
NP_F8 = ml_dtypes.float8_e4m3

_CACHE = {}

# V_DR: v projection as 12 fp8-DoubleRow matmuls (x8h@(wv8h+wv8lo) +
# x8lo@wv8h) instead of 8 bf16 ones. Model says DR streams 25% fewer
# cycles; hw pays per-instruction Ldweights the model doesn't count.
V_DR = True


def _emit_consts_pre(nc, consts, dram):
    """First-needed constants only: q/k weights + the mask-preload tiles.
    The bulkier wv8/wv8lo/wo DMAs are deferred past the rep-0 x loads so the
    first projection matmuls start as early as possible."""
    c = {}
    # identity + diag-block additive causal mask (M[p, j] = 0 if j >= p else
    # -1e30): preloaded into the diag psum block via one 128-col PE matmul so
    # exp writes exact zeros into the triangle and NOTHING downstream of the
    # exp ever blocks the PE stream.
    ident = consts.tile([128, 128], F32, name="ident")
    nc.gpsimd.memset(ident, 0.0)
    nc.gpsimd.affine_select(
        out=ident, in_=ident, compare_op=mybir.AluOpType.not_equal,
        fill=1.0, base=0, pattern=[[-1, 128]], channel_multiplier=1,
    )
    mband = consts.tile([128, 128], F32, name="mband")
    nc.gpsimd.memset(mband, 0.0)
    nc.gpsimd.affine_select(
        out=mband, in_=mband, compare_op=mybir.AluOpType.is_ge,
        fill=-1e30, base=0, pattern=[[1, 128]], channel_multiplier=-1,
    )
    c["identb"] = consts.tile([128, 128], BF16, name="identb")
    nc.vector.tensor_copy(c["identb"], ident)
    c["mb16"] = consts.tile([128, 128], BF16, name="mb16")
    nc.vector.tensor_copy(c["mb16"], mband)
    for key, shape, dt in (
        ("wk8", [128, 4, 2, JS], F8),
        ("wq8", [128, 4, 2, JS], F8),
        ("bq", [128, 2], F32),
    ):
        c[key] = consts.tile(shape, dt, name=key + "_sb")
        nc.sync.dma_start(out=c[key], in_=dram[key].ap())
    return c


def _emit_consts_rest(nc, c, consts, dram):
    keys = (
        (("wv8", [128, 4, 2, JS], F8), ("wv8lo", [128, 4, 2, JS], F8))
        if V_DR else (("wv", [128, ND, JS], BF16),)
    ) + (("wo", [128, 2, D], BF16),)
    for key, shape, dt in keys:
        c[key] = consts.tile(shape, dt, name=key + "_sb")
        nc.sync.dma_start(out=c[key], in_=dram[key].ap())


def _proj_fillers(nc, c, P, pools, dram, rep, tg, split_load=False):
    """One t-group's projections as small act-independent closures (~0.4-0.9us
    of PE each) to pump between attention chunks. Closure 0 issues the x
    loads (SP-triggered so no compute sequencer blocks on DMA sem waits).
    With split_load, the load is its own closure (rep-0 startup ordering)."""
    par = rep % 2
    qT, kT, vv = P[par]["qT"], P[par]["kT"], P[par]["vv"]
    xp, x8p, psP = pools["xt"], pools["x8"], pools["psP"]
    r = f"r{rep}"
    ts = slice(tg * 512, (tg + 1) * 512)
    st = {}

    def load():
        st["x8"] = x8p.tile([128, 4, 2, 512], F8, name=f"x8{r}_{tg}", tag="x8")
        nc.sync.dma_start(out=st["x8"], in_=dram["xT8"].ap()[:, :, :, ts])
        if V_DR:
            st["x8l"] = xp.tile(
                [128, 4, 2, 512], F8, name=f"x8l{r}_{tg}", tag="xl"
            )
            nc.sync.dma_start(out=st["x8l"], in_=dram["xT8lo"].ap()[:, :, :, ts])
        else:
            st["xt"] = xp.tile([128, ND, 512], BF16, name=f"xt{r}_{tg}", tag="xl")
            nc.sync.dma_start(out=st["xt"], in_=dram["xT"].ap()[:, :, ts])

    def qk(w8, b_sb, dstT, jc):
        def emit():
            ps = psP.tile([128, 512], F32, name=f"psqk{r}_{tg}", tag="pp")
            for c2 in range(4):
                nc.tensor.matmul(
                    ps,
                    w8[:, c2, :, jc * 128:(jc + 1) * 128],
                    st["x8"][:, c2, :, :],
                    start=(c2 == 0),
                    stop=(c2 == 3),
                    perf_mode=DR,
                )
            if b_sb is None:
                nc.vector.tensor_copy(out=dstT[:, jc, ts], in_=ps)
            else:
                nc.vector.tensor_scalar_add(
                    out=dstT[:, jc, ts], in0=ps, scalar1=b_sb[:, jc:jc + 1]
                )
        return emit

    def vchunk(i4):
        def emit():
            i = tg * 4 + i4
            psv = psP.tile([128, 512], F32, name=f"psv{r}_{i}", tag="pp")
            if V_DR:
                # v projection in fp8 DoubleRow, error-compensated with
                # three terms: x8h@(wv8h + wv8lo) + x8lo@wv8h (the dropped
                # lo@lo term is ~0.1%^2). Consecutive same-lhsT matmuls
                # keep the stationary weights.
                k = 0
                for c2 in range(4):
                    for lhs, rhs in (
                        (st["x8"], c["wv8"]),
                        (st["x8"], c["wv8lo"]),
                        (st["x8l"], c["wv8"]),
                    ):
                        nc.tensor.matmul(
                            psv[:, :JS],
                            lhs[:, c2, :, i4 * 128:(i4 + 1) * 128],
                            rhs[:, c2, :, :],
                            start=(k == 0),
                            stop=(k == 11),
                            perf_mode=DR,
                        )
                        k += 1
            else:
                for dc in range(ND):
                    nc.tensor.matmul(
                        psv[:, :JS],
                        st["xt"][:, dc, i4 * 128:(i4 + 1) * 128],
                        c["wv"][:, dc, :],
                        start=(dc == 0),
                        stop=(dc == ND - 1),
                    )
            nc.vector.tensor_copy(
                out=vv[:, :, i, 0:HD],
                in_=psv[:, :JS].rearrange("p (h e) -> p h e", h=HPC),
            )
        return emit

    def first():
        load()
        qk(c["wk8"], None, kT, 0)()

    rest = [
        qk(c["wk8"], None, kT, 1),
        qk(c["wq8"], c["bq"], qT, 0),
        qk(c["wq8"], c["bq"], qT, 1),
        vchunk(0),
        vchunk(1),
        vchunk(2),
        vchunk(3),
    ]
    if split_load:
        return [load, qk(c["wk8"], None, kT, 0)] + rest
    return [first] + rest


def _wo_fillers(nc, c, P, pools, dram, rep, irange):
    """Output projection + store for finished 128-row t-chunks, one closure
    per chunk (~0.85us of PE). Stores are f32 (the host sums f32 partials
    anyway, and skipping the bf16 round drops ~0.4% store rounding)."""
    par = rep % 2
    oT = P[par]["oT"]
    psP, obp = pools["psP"], pools["ob"]
    r = f"r{rep}"

    def one(i):
        def emit():
            for ng in range(2):
                ps = psP.tile([128, 512], F32, name=f"ps3{r}_{i}", tag="pp")
                for jc in range(2):
                    nc.tensor.matmul(
                        ps,
                        oT[:, jc, i * 128:(i + 1) * 128],
                        c["wo"][:, jc, ng * 512:(ng + 1) * 512],
                        start=(jc == 0),
                        stop=(jc == 1),
                    )
                ob = obp.tile([128, 512], F32, name=f"ob{r}_{i}", tag="ob")
                nc.vector.tensor_copy(ob, ps)
                nc.sync.dma_start(
                    out=dram["out"].ap()[
                        i * 128:(i + 1) * 128, ng * 512:(ng + 1) * 512
                    ],
                    in_=ob,
                )
        return emit

    return [one(i) for i in irange]


def _emit_head_gpair(nc, c, P, pools, rep, h, gset, fill=(), post_norm=None):
    """Scores + exp + p@v + normalize for one head over a pair of 512-wide
    query groups. `fill` closures (act-independent PE work) are pumped
    evenly between attention chunks to keep the PE busy while the Act
    engine's exp stream (the binding resource) drains.

    Masking costs no post-exp work: the causal triangle is preloaded as
    -1e30 into the diag psum block by one 128-col PE matmul, so exp writes
    exact zeros and p@v depends ONLY on the exp — nothing downstream of the
    exp ever blocks the PE stream."""
    par = rep % 2
    qT, kT, oT, vv = (P[par][k] for k in ("qT", "kT", "oT", "vv"))
    psS, psA, esb, nrm = pools["psS"], pools["psA"], pools["es"], pools["nrm"]
    r = f"r{rep}"
    jc, hr = h // 2, (h % 2) * 64

    accs = {
        g: psA.tile([128, 512], F32, name=f"acc{r}_{h}_{g}", tag="acc")
        for g in gset
    }
    pieces = []
    for ck in range(gset[-1] * 4 + 4):
        glist = [g for g in gset if ck <= 4 * g + 3]
        pieces.append((ck, glist))
    nch = len(pieces)
    fill = list(fill)
    pumped = 0
    ci = 0

    def pump():
        nonlocal pumped
        while pumped * nch < len(fill) * ci:
            fill[pumped]()
            pumped += 1

    def emit_pv(piece, es):
        ck, glist = piece
        for gi, g in enumerate(glist):
            junk = ck * 128 - g * 512
            glo = junk if junk > 0 else 0
            nc.tensor.matmul(
                accs[g][0:HD + 1, glo:512],
                vv[:, h, ck, 0:HD + 1],
                es[:, gi * 512 + glo:(gi + 1) * 512],
                start=(ck == 0),
                stop=(ck == 4 * g + 3),
            )

    def emit_norm(g):
        rc = nrm.tile([1, 512], F32, name=f"rc{r}_{h}_{g}", tag="rc")
        nc.vector.reciprocal(rc, accs[g][HD:HD + 1, :])
        rb = nrm.tile([64, 512], F32, name=f"rb{r}_{h}_{g}", tag="rb")
        nc.gpsimd.partition_broadcast(rb, rc)
        nc.vector.tensor_mul(
            oT[hr:hr + 64, jc, g * 512:(g + 1) * 512], accs[g][0:HD, :], rb
        )

    pending = []
    done_g = set()

    def flush_one():
        piece, es = pending.pop(0)
        emit_pv(piece, es)
        ck, glist = piece
        for g in glist:
            if ck == 4 * g + 3 and g not in done_g:
                done_g.add(g)
                emit_norm(g)
                if post_norm is not None:
                    post_norm(g)

    for ck, glist in pieces:
        width = len(glist) * 512
        ps = psS.tile([128, width], F32, name=f"psrow{r}_{h}", tag="ps")
        lo = 0
        for gi, g in enumerate(glist):
            junk = ck * 128 - g * 512
            diag = junk >= 0  # only ever at gi == 0
            kslice = kT[hr:hr + 64, jc, ck * 128:ck * 128 + 128]
            if diag:
                lo = junk
                # -1e30 triangle preload on the diag 128-block (start=True
                # lazily zeroes the whole psum region), then the scores
                # matmul accumulates onto it: exp writes exact zeros into
                # the non-causal triangle.
                nc.tensor.matmul(
                    ps[:, junk:junk + 128],
                    c["identb"],
                    c["mb16"],
                    start=True,
                    stop=False,
                )
                nc.tensor.matmul(
                    ps[:, junk:512],
                    kslice,
                    qT[hr:hr + 64, jc, g * 512 + junk:(g + 1) * 512],
                    start=False,
                    stop=True,
                )
                continue
            nc.tensor.matmul(
                ps[:, gi * 512:(gi + 1) * 512],
                kslice,
                qT[hr:hr + 64, jc, g * 512:g * 512 + 512],
                start=True,
                stop=True,
            )
        es = esb.tile([128, 1024], BF16, name=f"es{r}_{h}", tag="es")
        nc.scalar.activation(es[:, lo:width], ps[:, lo:width], EXP, scale=SCALE_EXP)
        ci += 1
        pump()
        pending.append(((ck, glist), es))
        if len(pending) > 4:
            flush_one()
    while pending:
        flush_one()


def _interleave(a, b):
    out, a, b = [], list(a), list(b)
    while a or b:
        if a:
            out.append(a.pop(0))
        if b:
            out.append(b.pop(0))
    return out


def _emit_body(nc, c, P, pools, dram, rep, reps, consts):
    first, last = rep == 0, rep == reps - 1
    args = (nc, c, P, pools, dram)
    if first:
        # startup: x loads for tg0/1 are issued BEFORE the bulky wv8/wo
        # const DMAs so the first projection matmuls start ~early.
        f0 = _proj_fillers(*args, 0, 0, split_load=True)
        f1 = _proj_fillers(*args, 0, 1, split_load=True)
        f0[0]()
        f1[0]()
        _emit_consts_rest(nc, c, consts, dram)
        for f in f0[1:] + f1[1:]:
            f()
    # phase balance (per-rep): gset(0,1) heads carry ~21us of exp, so they
    # get only the tg2/3 projections (~11us of PE filler); gset(2,3) heads
    # carry ~55us of exp and absorb everything else (~24us of filler).
    f01 = _proj_fillers(*args, rep, 2) + _proj_fillers(*args, rep, 3)
    tails = [] if first else _wo_fillers(*args, rep - 1, range(8, 16))
    wos = _wo_fillers(*args, rep, range(0, 8))
    nxt = [] if last else (
        _proj_fillers(*args, rep + 1, 0) + _proj_fillers(*args, rep + 1, 1)
    )
    f23 = _interleave(nxt, tails + wos)
    p01 = [f01[(len(f01) * i) // HPC:(len(f01) * (i + 1)) // HPC]
           for i in range(HPC)]
    p23 = [f23[(len(f23) * i) // HPC:(len(f23) * (i + 1)) // HPC]
           for i in range(HPC)]
    for h in range(HPC):
        _emit_head_gpair(nc, c, P, pools, rep, h, (0, 1), fill=p01[h])
    for h in range(HPC):
        post_norm = None
        if last and h == HPC - 1:
            # drain shortening: the last rep's tail out-projection chunks
            # fire as soon as their group's final norm lands (all other
            # heads' norms for that group are already done by h==3).
            tail_wo = {
                2: _wo_fillers(*args, rep, range(8, 12)),
                3: _wo_fillers(*args, rep, range(12, 16)),
            }

            def post_norm(g, tail_wo=tail_wo):
                for f in tail_wo.pop(g, []):
                    f()

        _emit_head_gpair(nc, c, P, pools, rep, h, (2, 3), fill=p23[h],
                         post_norm=post_norm)


def build(reps=1):
    nc = bacc.Bacc("TRN2", target_bir_lowering=False, num_devices=NCORES)
    dram = {
        "xT8": nc.dram_tensor("xT8", [128, 4, 2, T], F8, kind="ExternalInput"),
        "wq8": nc.dram_tensor("wq8", [128, 4, 2, JS], F8, kind="ExternalInput"),
        "wk8": nc.dram_tensor("wk8", [128, 4, 2, JS], F8, kind="ExternalInput"),
        "wo": nc.dram_tensor("wo", [128, 2, D], BF16, kind="ExternalInput"),
        "bq": nc.dram_tensor("bq", [128, 2], F32, kind="ExternalInput"),
        "out": nc.dram_tensor("out", [T, D], F32, kind="ExternalOutput"),
    }
    if V_DR:
        dram["xT8lo"] = nc.dram_tensor(
            "xT8lo", [128, 4, 2, T], F8, kind="ExternalInput"
        )
        dram["wv8"] = nc.dram_tensor(
            "wv8", [128, 4, 2, JS], F8, kind="ExternalInput"
        )
        dram["wv8lo"] = nc.dram_tensor(
            "wv8lo", [128, 4, 2, JS], F8, kind="ExternalInput"
        )
    else:
        dram["xT"] = nc.dram_tensor("xT", [128, ND, T], BF16, kind="ExternalInput")
        dram["wv"] = nc.dram_tensor("wv", [128, ND, JS], BF16, kind="ExternalInput")
    with tile.TileContext(nc) as tc:
        with (
            tc.tile_pool(name="consts", bufs=1) as consts,
            tc.tile_pool(name="persist", bufs=1) as persistp,
            tc.tile_pool(name="xt", bufs=3) as xp,
            tc.tile_pool(name="x8", bufs=3) as x8p,
            tc.tile_pool(name="psP", bufs=2, space="PSUM") as psP,
            tc.tile_pool(name="psS", bufs=2, space="PSUM") as psS,
            tc.tile_pool(name="psA", bufs=2, space="PSUM") as psA,
            tc.tile_pool(name="nrm", bufs=3) as nrm,
            tc.tile_pool(name="ob", bufs=6) as obp,
            tc.tile_pool(name="es", bufs=8) as esb,
        ):
            c = _emit_consts_pre(nc, consts, dram)
            P = {}
            for par in range(2):
                P[par] = {
                    "qT": persistp.tile([128, 2, T], BF16, name=f"qT_{par}"),
                    "kT": persistp.tile([128, 2, T], BF16, name=f"kT_{par}"),
                    "vv": persistp.tile(
                        [128, HPC, NT, HD + 2], BF16, name=f"vv_{par}"
                    ),
                    "oT": persistp.tile([128, 2, T], BF16, name=f"oT_{par}"),
                }
                # denominator row: 65th column of v is a constant matching
                # v's scale (wv is prescaled by 8 for fp8 range in the DR
                # path), so the normalize cancels the scale for free.
                nc.gpsimd.memset(
                    P[par]["vv"][:, :, :, HD:HD + 1], WS if V_DR else 1.0
                )
            pools = {
                "xt": xp, "x8": x8p, "psP": psP, "psS": psS, "psA": psA,
                "nrm": nrm, "es": esb, "ob": obp,
            }
            for rep in range(reps):
                _emit_body(nc, c, P, pools, dram, rep, reps, consts)
    nc.compile()
    return nc


def _prep_core(x_b, wq, bq, wk, bk, wv, bv, wo, js):
    """Host-side shard + relayout + cast for one core."""
    f32 = np.float32
    xT = np.ascontiguousarray(x_b.T)  # [D, T], row d = dc*128+p

    def lay8(a):  # [D, N] -> [128, 4, 2, N] fp8 DoubleRow layout
        return np.ascontiguousarray(
            a.reshape(4, 2, 128, -1).transpose(2, 0, 1, 3).astype(NP_F8)
        )

    xT8h = xT.astype(NP_F8).astype(f32)
    xT8 = lay8(xT)
    xT8lo = lay8(xT - xT8h)

    def w8_pair(w):
        wp = (WS * w[:, js]).astype(f32)
        hi = wp.astype(NP_F8).astype(f32)
        return lay8(wp), lay8(wp - hi)

    def qk_b(b):
        bp = (WS * b[js]).astype(f32)
        return np.ascontiguousarray(bp.reshape(2, 128).T)

    woc = np.ascontiguousarray(
        wo[js, :].reshape(2, 2, HD, D).transpose(1, 2, 0, 3)
        .reshape(128, 2, D).astype(NP_BF16)
    )
    wv8, wv8lo = w8_pair(wv)
    return {
        "xT8": xT8,
        "xT8lo": xT8lo,
        "xT": np.ascontiguousarray(
            xT.reshape(ND, 128, T).transpose(1, 0, 2).astype(NP_BF16)
        ),
        "wq8": w8_pair(wq)[0],
        "wk8": w8_pair(wk)[0],
        "wv8": wv8,
        "wv8lo": wv8lo,
        "wv": np.ascontiguousarray(
            wv[:, js].reshape(ND, 128, JS).transpose(1, 0, 2).astype(NP_BF16)
        ),
        "wo": woc,
        "bq": qk_b(bq),
    }


def _in_maps(inputs):
    f32 = np.float32
    x = np.asarray(inputs["x"], f32)
    wq = np.asarray(inputs["wq"], f32)
    bq = np.asarray(inputs["bq"], f32)
    wk = np.asarray(inputs["wk"], f32)
    bk = np.asarray(inputs["bk"], f32)
    wv = np.asarray(inputs["wv"], f32)
    bv = np.asarray(inputs["bv"], f32)
    wo = np.asarray(inputs["wo"], f32)
    maps = []
    for cc in range(NCORES):
        b, g = cc // HPC, cc % HPC
        js = slice(g * JS, (g + 1) * JS)
        maps.append(_prep_core(x[b], wq, bq, wk, bk, wv, bv, wo, js))
    return maps


def kernel(**inputs) -> np.ndarray:
    from concourse.bass_utils import run_bass_kernel_spmd

    if "nc" not in _CACHE:
        _CACHE["nc"] = build()
    nc = _CACHE["nc"]
    maps = _in_maps(inputs)
    res = run_bass_kernel_spmd(nc, maps, core_ids=list(range(NCORES)))
    out = np.zeros((B, T, D), dtype=np.float32)
    for cc in range(NCORES):
        out[cc // HPC] += np.asarray(res.results[cc]["out"], dtype=np.float32)
    # bv is excluded on-device (softmax rows sum to 1 => attn@(1 x bv)@wo is
    # the constant row bv@wo) and bk is dropped (softmax shift invariance):
    # fold bv@wo into bo here.
    bo_eff = np.asarray(inputs["bo"], np.float32) + (
        np.asarray(inputs["bv"], np.float32) @ np.asarray(inputs["wo"], np.float32)
    )
    out += bo_eff[None, None, :]
    return out


# revision 57
# speedup vs baseline: 1.0413x; 1.0413x over previous
"""Causal multi-head attention (B=2, T=2048, D=1024, H=16) on 8 trn2 cores.

Sharding: data-parallel over batch (2) x tensor-parallel over heads (4 groups
of 4 heads): core c handles batch c//4, head group c%4. Each core computes
q/k/v projections for its 256 feature columns, causal attention for its 4
heads, and a partial row-parallel output projection. The host sums the 4
partials per batch and adds bo (plus bv@wo, see below).

Numerics/layout strategy (measured rel err 1.603e-2 vs the f32 reference,
HW exec ~97us/rep by repeat-slope, down from the 110.7us baseline):
- Host pre-transposes x to d-major and pre-casts fp8-e4m3: xT8 plus the
  residual xT8lo, so the device does zero transposes.
- Q/K projections run as fp8 DoubleRow matmuls (256-deep contraction).
  Weights are pre-scaled by 8 on the host (folded back out of the softmax
  exp scale) to keep fp8 away from the subnormal range. The V projection
  runs fp8 DoubleRow too, error-compensated with three terms
  (x8@(wv8+wv8lo) + x8lo@wv8, ~0.1% error); the 8x weight prescale cancels
  in the softmax normalize because the denominator ones-column is set to 8.
  q/k/v are stored bf16; scores and p@v run bf16 -> fp32 psum. (fp8 for the
  exp'd scores themselves was measured at +2.3% output error - over the
  gate - so p@v stays bf16.)
- bk is dropped entirely: (q+bq)@(k+bk) differs from (q+bq)@k by a
  per-query constant, which softmax is invariant to.
- bv is dropped on-device: normalized attention rows sum to 1, so
  attn@(1 x bv)@wo == bv@wo, a constant row the host folds into bo.
- The causal mask costs nothing downstream of exp: a single 128-col PE
  matmul (identb @ mb16) preloads -1e30 into the diag psum triangle before
  the scores accumulate, so exp writes exact zeros and p@v depends ONLY on
  the exp. Diagonal matmuls skip the non-causal junk columns.
- The exp stream on the scalar (Act) engine is a ~76us/rep floor; the PE is
  the binding engine at ~93us/rep. All act-independent PE work (q/k/v
  projections, output projection) is cut into ~0.5-0.9us closures and
  pumped between attention chunks so neither engine starves: tg2/3
  projections fill the (0,1)-group heads; the previous rep's tail
  out-projection, this rep's rows-0:1024 out-projection and the NEXT rep's
  tg0/1 projections fill the (2,3)-group heads. On the last rep the tail
  out-projection fires via post_norm hooks as each group's final norm
  lands, shortening the drain.
- DMA loads are SP-triggered (a dma_start's sem waits hold the issuing
  sequencer; on Act that would head-of-line block the exp stream). Output
  stores stage through SBUF in f32 on the DVE (GPSIMD cannot touch PSUM and
  DMA cannot read it) - f32 stores also drop the bf16 store rounding.
- Startup is ordered x8(tg0/1) + q/k weights first, bulky wv8/wo consts
  after, so the first projection matmuls start ~3us in. Persistent state is
  parity-double-buffered so consecutive reps pipeline.
"""

import sys

if "/opt/trn_rl_repo" not in sys.path:
    sys.path.insert(0, "/opt/trn_rl_repo")

import numpy as np
import ml_dtypes

import concourse.bass as bass
import concourse.mybir as mybir
import concourse.tile as tile
from concourse import bacc

F32 = mybir.dt.float32
BF16 = mybir.dt.bfloat16
F8 = mybir.dt.float8e4
EXP = mybir.ActivationFunctionType.Exp
DR = mybir.MatmulPerfMode.DoubleRow

B, T, D, H, HD = 2, 2048, 1024, 16, 64
SCALE = float(D) ** -0.5  # module scales by d_model^-0.5
NCORES = 8
HPC = 4  # heads per core
JS = HPC * HD  # 256 feature columns per core
NT = T // 128  # 16 t-chunks
ND = D // 128  # 8 d-chunks
WS = 8.0  # fp8 weight prescale, folded out of the exp scale
SCALE_EXP = SCALE / (WS * WS)

NP_BF16 = ml_dtypes.bfloat16
NP_F8 = ml_dtypes.float8_e4m3

_CACHE = {}

# V_DR: v projection as 12 fp8-DoubleRow matmuls (x8h@(wv8h+wv8lo) +
# x8lo@wv8h) instead of 8 bf16 ones. Model says DR streams 25% fewer
# cycles; hw pays per-instruction Ldweights the model doesn't count.
V_DR = True


def _emit_consts_pre(nc, consts, dram):
    """First-needed constants only: q/k weights + the mask-preload tiles.
    The bulkier wv8/wv8lo/wo DMAs are deferred past the rep-0 x loads so the
    first projection matmuls start as early as possible."""
    c = {}
    # identity + diag-block additive causal mask (M[p, j] = 0 if j >= p else
    # -1e30): preloaded into the diag psum block via one 128-col PE matmul so
    # exp writes exact zeros into the triangle and NOTHING downstream of the
    # exp ever blocks the PE stream.
    ident = consts.tile([128, 128], F32, name="ident")
    nc.gpsimd.memset(ident, 0.0)
    nc.gpsimd.affine_select(
        out=ident, in_=ident, compare_op=mybir.AluOpType.not_equal,
        fill=1.0, base=0, pattern=[[-1, 128]], channel_multiplier=1,
    )
    mband = consts.tile([128, 128], F32, name="mband")
    nc.gpsimd.memset(mband, 0.0)
    nc.gpsimd.affine_select(
        out=mband, in_=mband, compare_op=mybir.AluOpType.is_ge,
        fill=-1e30, base=0, pattern=[[1, 128]], channel_multiplier=-1,
    )
    c["identb"] = consts.tile([128, 128], BF16, name="identb")
    nc.vector.tensor_copy(c["identb"], ident)
    c["mb16"] = consts.tile([128, 128], BF16, name="mb16")
    nc.vector.tensor_copy(c["mb16"], mband)
    for key, shape, dt in (
        ("wk8", [128, 4, 2, JS], F8),
        ("wq8", [128, 4, 2, JS], F8),
        ("bq", [128, 2], F32),
    ):
        c[key] = consts.tile(shape, dt, name=key + "_sb")
        nc.sync.dma_start(out=c[key], in_=dram[key].ap())
    return c


def _emit_consts_rest(nc, c, consts, dram):
    keys = (
        (("wv8", [128, 4, 2, JS], F8), ("wv8lo", [128, 4, 2, JS], F8))
        if V_DR else (("wv", [128, ND, JS], BF16),)
    ) + (("wo", [128, 2, D], BF16),)
    for key, shape, dt in keys:
        c[key] = consts.tile(shape, dt, name=key + "_sb")
        nc.sync.dma_start(out=c[key], in_=dram[key].ap())


def _proj_fillers(nc, c, P, pools, dram, rep, tg, split_load=False):
    """One t-group's projections as small act-independent closures (~0.4-0.9us
    of PE each) to pump between attention chunks. Closure 0 issues the x
    loads (SP-triggered so no compute sequencer blocks on DMA sem waits).
    With split_load, the load is its own closure (rep-0 startup ordering)."""
    par = rep % 2
    qT, kT, vv = P[par]["qT"], P[par]["kT"], P[par]["vv"]
    xp, x8p, psP = pools["xt"], pools["x8"], pools["psP"]
    r = f"r{rep}"
    ts = slice(tg * 512, (tg + 1) * 512)
    st = {}

    def load():
        st["x8"] = x8p.tile([128, 4, 2, 512], F8, name=f"x8{r}_{tg}", tag="x8")
        nc.sync.dma_start(out=st["x8"], in_=dram["xT8"].ap()[:, :, :, ts])
        if V_DR:
            st["x8l"] = xp.tile(
                [128, 4, 2, 512], F8, name=f"x8l{r}_{tg}", tag="xl"
            )
            nc.sync.dma_start(out=st["x8l"], in_=dram["xT8lo"].ap()[:, :, :, ts])
        else:
            st["xt"] = xp.tile([128, ND, 512], BF16, name=f"xt{r}_{tg}", tag="xl")
            nc.sync.dma_start(out=st["xt"], in_=dram["xT"].ap()[:, :, ts])

    def qk(w8, b_sb, dstT, jc):
        def emit():
            ps = psP.tile([128, 512], F32, name=f"psqk{r}_{tg}", tag="pp")
            for c2 in range(4):
                nc.tensor.matmul(
                    ps,
                    w8[:, c2, :, jc * 128:(jc + 1) * 128],
                    st["x8"][:, c2, :, :],
                    start=(c2 == 0),
                    stop=(c2 == 3),
                    perf_mode=DR,
                )
            if b_sb is None:
                nc.vector.tensor_copy(out=dstT[:, jc, ts], in_=ps)
            else:
                nc.vector.tensor_scalar_add(
                    out=dstT[:, jc, ts], in0=ps, scalar1=b_sb[:, jc:jc + 1]
                )
        return emit

    def vchunk(i4):
        def emit():
            i = tg * 4 + i4
            psv = psP.tile([128, 512], F32, name=f"psv{r}_{i}", tag="pp")
            if V_DR:
                # v projection in fp8 DoubleRow, error-compensated with
                # three terms: x8h@(wv8h + wv8lo) + x8lo@wv8h (the dropped
                # lo@lo term is ~0.1%^2). Consecutive same-lhsT matmuls
                # keep the stationary weights.
                k = 0
                for c2 in range(4):
                    for lhs, rhs in (
                        (st["x8"], c["wv8"]),
                        (st["x8"], c["wv8lo"]),
                        (st["x8l"], c["wv8"]),
                    ):
                        nc.tensor.matmul(
                            psv[:, :JS],
                            lhs[:, c2, :, i4 * 128:(i4 + 1) * 128],
                            rhs[:, c2, :, :],
                            start=(k == 0),
                            stop=(k == 11),
                            perf_mode=DR,
                        )
                        k += 1
            else:
                for dc in range(ND):
                    nc.tensor.matmul(
                        psv[:, :JS],
                        st["xt"][:, dc, i4 * 128:(i4 + 1) * 128],
                        c["wv"][:, dc, :],
                        start=(dc == 0),
                        stop=(dc == ND - 1),
                    )
            nc.vector.tensor_copy(
                out=vv[:, :, i, 0:HD],
                in_=psv[:, :JS].rearrange("p (h e) -> p h e", h=HPC),
            )
        return emit

    def first():
        load()
        qk(c["wk8"], None, kT, 0)()

    rest = [
        qk(c["wk8"], None, kT, 1),
        qk(c["wq8"], c["bq"], qT, 0),
        qk(c["wq8"], c["bq"], qT, 1),
        vchunk(0),
        vchunk(1),
        vchunk(2),
        vchunk(3),
    ]
    if split_load:
        return [load, qk(c["wk8"], None, kT, 0)] + rest
    return [first] + rest


def _wo_fillers(nc, c, P, pools, dram, rep, irange):
    """Output projection + store for finished 128-row t-chunks, one closure
    per chunk (~0.85us of PE). Stores are f32 (the host sums f32 partials
    anyway, and skipping the bf16 round drops ~0.4% store rounding)."""
    par = rep % 2
    oT = P[par]["oT"]
    psP, obp = pools["psP"], pools["ob"]
    r = f"r{rep}"

    def one(i):
        def emit():
            for ng in range(2):
                ps = psP.tile([128, 512], F32, name=f"ps3{r}_{i}", tag="pp")
                for jc in range(2):
                    nc.tensor.matmul(
                        ps,
                        oT[:, jc, i * 128:(i + 1) * 128],
                        c["wo"][:, jc, ng * 512:(ng + 1) * 512],
                        start=(jc == 0),
                        stop=(jc == 1),
                    )
                ob = obp.tile([128, 512], F32, name=f"ob{r}_{i}", tag="ob")
                nc.vector.tensor_copy(ob, ps)
                nc.sync.dma_start(
                    out=dram["out"].ap()[
                        i * 128:(i + 1) * 128, ng * 512:(ng + 1) * 512
                    ],
                    in_=ob,
                )
        return emit

    return [one(i) for i in irange]


def _emit_head_gpair(nc, c, P, pools, rep, h, gset, fill=(), post_norm=None):
    """Scores + exp + p@v + normalize for one head over a pair of 512-wide
    query groups. `fill` closures (act-independent PE work) are pumped
    evenly between attention chunks to keep the PE busy while the Act
    engine's exp stream (the binding resource) drains.

    Masking costs no post-exp work: the causal triangle is preloaded as
    -1e30 into the diag psum block by one 128-col PE matmul, so exp writes
    exact zeros and p@v depends ONLY on the exp — nothing downstream of the
    exp ever blocks the PE stream."""
    par = rep % 2
    qT, kT, oT, vv = (P[par][k] for k in ("qT", "kT", "oT", "vv"))
    psS, psA, esb, nrm = pools["psS"], pools["psA"], pools["es"], pools["nrm"]
    r = f"r{rep}"
    jc, hr = h // 2, (h % 2) * 64

    accs = {
        g: psA.tile([128, 512], F32, name=f"acc{r}_{h}_{g}", tag="acc")
        for g in gset
    }
    pieces = []
    for ck in range(gset[-1] * 4 + 4):
        glist = [g for g in gset if ck <= 4 * g + 3]
        pieces.append((ck, glist))
    nch = len(pieces)
    fill = list(fill)
    pumped = 0
    ci = 0

    def pump():
        nonlocal pumped
        while pumped * nch < len(fill) * ci:
            fill[pumped]()
            pumped += 1

    def emit_pv(piece, es):
        ck, glist = piece
        for gi, g in enumerate(glist):
            junk = ck * 128 - g * 512
            glo = junk if junk > 0 else 0
            nc.tensor.matmul(
                accs[g][0:HD + 1, glo:512],
                vv[:, h, ck, 0:HD + 1],
                es[:, gi * 512 + glo:(gi + 1) * 512],
                start=(ck == 0),
                stop=(ck == 4 * g + 3),
            )

    def emit_norm(g):
        rc = nrm.tile([1, 512], F32, name=f"rc{r}_{h}_{g}", tag="rc")
        nc.vector.reciprocal(rc, accs[g][HD:HD + 1, :])
        rb = nrm.tile([64, 512], F32, name=f"rb{r}_{h}_{g}", tag="rb")
        nc.gpsimd.partition_broadcast(rb, rc)
        nc.vector.tensor_mul(
            oT[hr:hr + 64, jc, g * 512:(g + 1) * 512], accs[g][0:HD, :], rb
        )

    pending = []
    done_g = set()

    def flush_one():
        piece, es = pending.pop(0)
        emit_pv(piece, es)
        ck, glist = piece
        for g in glist:
            if ck == 4 * g + 3 and g not in done_g:
                done_g.add(g)
                emit_norm(g)
                if post_norm is not None:
                    post_norm(g)

    for ck, glist in pieces:
        width = len(glist) * 512
        ps = psS.tile([128, width], F32, name=f"psrow{r}_{h}", tag="ps")
        lo = 0
        for gi, g in enumerate(glist):
            junk = ck * 128 - g * 512
            diag = junk >= 0  # only ever at gi == 0
            kslice = kT[hr:hr + 64, jc, ck * 128:ck * 128 + 128]
            if diag:
                lo = junk
                # -1e30 triangle preload on the diag 128-block (start=True
                # lazily zeroes the whole psum region), then the scores
                # matmul accumulates onto it: exp writes exact zeros into
                # the non-causal triangle.
                nc.tensor.matmul(
                    ps[:, junk:junk + 128],
                    c["identb"],
                    c["mb16"],
                    start=True,
                    stop=False,
                )
                nc.tensor.matmul(
                    ps[:, junk:512],
                    kslice,
                    qT[hr:hr + 64, jc, g * 512 + junk:(g + 1) * 512],
                    start=False,
                    stop=True,
                )
                continue
            nc.tensor.matmul(
                ps[:, gi * 512:(gi + 1) * 512],
                kslice,
                qT[hr:hr + 64, jc, g * 512:g * 512 + 512],
                start=True,
                stop=True,
            )
        es = esb.tile([128, 1024], BF16, name=f"es{r}_{h}", tag="es")
        nc.scalar.activation(es[:, lo:width], ps[:, lo:width], EXP, scale=SCALE_EXP)
        ci += 1
        pump()
        pending.append(((ck, glist), es))
        if len(pending) > 4:
            flush_one()
    while pending:
        flush_one()


def _interleave(a, b):
    out, a, b = [], list(a), list(b)
    while a or b:
        if a:
            out.append(a.pop(0))
        if b:
            out.append(b.pop(0))
    return out


def _emit_body(nc, c, P, pools, dram, rep, reps, consts):
    first, last = rep == 0, rep == reps - 1
    args = (nc, c, P, pools, dram)
    if first:
        # startup: x loads for tg0/1 are issued BEFORE the bulky wv8/wo
        # const DMAs so the first projection matmuls start ~early.
        f0 = _proj_fillers(*args, 0, 0, split_load=True)
        f1 = _proj_fillers(*args, 0, 1, split_load=True)
        f0[0]()
        f1[0]()
        _emit_consts_rest(nc, c, consts, dram)
        for f in f0[1:] + f1[1:]:
            f()
    # phase balance (per-rep): gset(0,1) heads carry ~21us of exp, so they
    # get only the tg2/3 projections (~11us of PE filler); gset(2,3) heads
    # carry ~55us of exp and absorb everything else (~24us of filler).
    f01 = _proj_fillers(*args, rep, 2) + _proj_fillers(*args, rep, 3)
    tails = [] if first else _wo_fillers(*args, rep - 1, range(8, 16))
    wos = _wo_fillers(*args, rep, range(0, 8))
    nxt = [] if last else (
        _proj_fillers(*args, rep + 1, 0) + _proj_fillers(*args, rep + 1, 1)
    )
    f23 = _interleave(nxt, tails + wos)
    p01 = [f01[(len(f01) * i) // HPC:(len(f01) * (i + 1)) // HPC]
           for i in range(HPC)]
    p23 = [f23[(len(f23) * i) // HPC:(len(f23) * (i + 1)) // HPC]
           for i in range(HPC)]
    for h in range(HPC):
        _emit_head_gpair(nc, c, P, pools, rep, h, (0, 1), fill=p01[h])
    for h in range(HPC):
        post_norm = None
        if last and h == HPC - 1:
            # drain shortening: the last rep's tail out-projection chunks
            # fire as soon as their group's final norm lands (all other
            # heads' norms for that group are already done by h==3).
            tail_wo = {
                2: _wo_fillers(*args, rep, range(8, 12)),
                3: _wo_fillers(*args, rep, range(12, 16)),
            }

            def post_norm(g, tail_wo=tail_wo):
                for f in tail_wo.pop(g, []):
                    f()

        _emit_head_gpair(nc, c, P, pools, rep, h, (2, 3), fill=p23[h],
                         post_norm=post_norm)


def build(reps=1):
    nc = bacc.Bacc("TRN2", target_bir_lowering=False, num_devices=NCORES)
    dram = {
        "xT8": nc.dram_tensor("xT8", [128, 4, 2, T], F8, kind="ExternalInput"),
        "wq8": nc.dram_tensor("wq8", [128, 4, 2, JS], F8, kind="ExternalInput"),
        "wk8": nc.dram_tensor("wk8", [128, 4, 2, JS], F8, kind="ExternalInput"),
        "wo": nc.dram_tensor("wo", [128, 2, D], BF16, kind="ExternalInput"),
        "bq": nc.dram_tensor("bq", [128, 2], F32, kind="ExternalInput"),
        "out": nc.dram_tensor("out", [T, D], F32, kind="ExternalOutput"),
    }
    if V_DR:
        dram["xT8lo"] = nc.dram_tensor(
            "xT8lo", [128, 4, 2, T], F8, kind="ExternalInput"
        )
        dram["wv8"] = nc.dram_tensor(
            "wv8", [128, 4, 2, JS], F8, kind="ExternalInput"
        )
        dram["wv8lo"] = nc.dram_tensor(
            "wv8lo", [128, 4, 2, JS], F8, kind="ExternalInput"
        )
    else:
        dram["xT"] = nc.dram_tensor("xT", [128, ND, T], BF16, kind="ExternalInput")
        dram["wv"] = nc.dram_tensor("wv", [128, ND, JS], BF16, kind="ExternalInput")
    with tile.TileContext(nc) as tc:
        with (
            tc.tile_pool(name="consts", bufs=1) as consts,
            tc.tile_pool(name="persist", bufs=1) as persistp,
            tc.tile_pool(name="xt", bufs=3) as xp,
            tc.tile_pool(name="x8", bufs=3) as x8p,
            tc.tile_pool(name="psP", bufs=2, space="PSUM") as psP,
            tc.tile_pool(name="psS", bufs=2, space="PSUM") as psS,
            tc.tile_pool(name="psA", bufs=2, space="PSUM") as psA,
            tc.tile_pool(name="nrm", bufs=3) as nrm,
            tc.tile_pool(name="ob", bufs=6) as obp,
            tc.tile_pool(name="es", bufs=8) as esb,
        ):
            c = _emit_consts_pre(nc, consts, dram)
            P = {}
            for par in range(2):
                P[par] = {
                    "qT": persistp.tile([128, 2, T], BF16, name=f"qT_{par}"),
                    "kT": persistp.tile([128, 2, T], BF16, name=f"kT_{par}"),
                    "vv": persistp.tile(
                        [128, HPC, NT, HD + 2], BF16, name=f"vv_{par}"
                    ),
                    "oT": persistp.tile([128, 2, T], BF16, name=f"oT_{par}"),
                }
                # denominator row: 65th column of v is a constant matching
                # v's scale (wv is prescaled by 8 for fp8 range in the DR
                # path), so the normalize cancels the scale for free.
                nc.gpsimd.memset(
                    P[par]["vv"][:, :, :, HD:HD + 1], WS if V_DR else 1.0
                )
            pools = {
                "xt": xp, "x8": x8p, "psP": psP, "psS": psS, "psA": psA,
                "nrm": nrm, "es": esb, "ob": obp,
            }
            for rep in range(reps):
                _emit_body(nc, c, P, pools, dram, rep, reps, consts)
    nc.compile()
    return nc


def _prep_core(x_b, wq, bq, wk, bk, wv, bv, wo, js):
    """Host-side shard + relayout + cast for one core."""
    f32 = np.float32
    xT = np.ascontiguousarray(x_b.T)  # [D, T], row d = dc*128+p

    def lay8(a):  # [D, N] -> [128, 4, 2, N] fp8 DoubleRow layout
        return np.ascontiguousarray(
            a.reshape(4, 2, 128, -1).transpose(2, 0, 1, 3).astype(NP_F8)
        )

    xT8h = xT.astype(NP_F8).astype(f32)
    xT8 = lay8(xT)
    xT8lo = lay8(xT - xT8h)

    def w8_pair(w):
        wp = (WS * w[:, js]).astype(f32)
        hi = wp.astype(NP_F8).astype(f32)
        return lay8(wp), lay8(wp - hi)

    def qk_b(b):
        bp = (WS * b[js]).astype(f32)
        return np.ascontiguousarray(bp.reshape(2, 128).T)

    woc = np.ascontiguousarray(
        wo[js, :].reshape(2, 2, HD, D).transpose(1, 2, 0, 3)
        .reshape(128, 2, D).astype(NP_BF16)
    )
    wv8, wv8lo = w8_pair(wv)
    m = {
        "xT8": xT8,
        "wq8": w8_pair(wq)[0],
        "wk8": w8_pair(wk)[0],
        "wo": woc,
        "bq": qk_b(bq),
    }
    if V_DR:
        m["xT8lo"] = xT8lo
        m["wv8"] = wv8
        m["wv8lo"] = wv8lo
    else:
        m["xT"] = np.ascontiguousarray(
            xT.reshape(ND, 128, T).transpose(1, 0, 2).astype(NP_BF16)
        )
        m["wv"] = np.ascontiguousarray(
            wv[:, js].reshape(ND, 128, JS).transpose(1, 0, 2).astype(NP_BF16)
        )
    return m


def _in_maps(inputs):
    f32 = np.float32
    x = np.asarray(inputs["x"], f32)
    wq = np.asarray(inputs["wq"], f32)
    bq = np.asarray(inputs["bq"], f32)
    wk = np.asarray(inputs["wk"], f32)
    bk = np.asarray(inputs["bk"], f32)
    wv = np.asarray(inputs["wv"], f32)
    bv = np.asarray(inputs["bv"], f32)
    wo = np.asarray(inputs["wo"], f32)
    maps = []
    for cc in range(NCORES):
        b, g = cc // HPC, cc % HPC
        js = slice(g * JS, (g + 1) * JS)
        maps.append(_prep_core(x[b], wq, bq, wk, bk, wv, bv, wo, js))
    return maps


def kernel(**inputs) -> np.ndarray:
    from concourse.bass_utils import run_bass_kernel_spmd

    if "nc" not in _CACHE:
        _CACHE["nc"] = build()
    nc = _CACHE["nc"]
    maps = _in_maps(inputs)
    res = run_bass_kernel_spmd(nc, maps, core_ids=list(range(NCORES)))
    out = np.zeros((B, T, D), dtype=np.float32)
    for cc in range(NCORES):
        out[cc // HPC] += np.asarray(res.results[cc]["out"], dtype=np.float32)
    # bv is excluded on-device (softmax rows sum to 1 => attn@(1 x bv)@wo is
    # the constant row bv@wo) and bk is dropped (softmax shift invariance):
    # fold bv@wo into bo here.
    bo_eff = np.asarray(inputs["bo"], np.float32) + (
        np.asarray(inputs["bv"], np.float32) @ np.asarray(inputs["wo"], np.float32)
    )
    out += bo_eff[None, None, :]
    return out
